# revision 29
# baseline (speedup 1.0000x reference)
"""GroupQuantLinear on 8 Trainium2 NeuronCores.

y[b,s,o] = x[b,s,:] @ W[o,:] + bias[o], where W is dequantized on-device from
4-bit packed weights with per-(o, group) affine scale/bias (groups of 256 along
the 4096-wide input dim).

Sharding: tensor-parallel on out_features (8 shards of 2048 rows); x replicated.

Per-core kernel (Bass/Tile), transpose-free, W-stationary design:
  The packed words are transposed on the HOST (layout-only) to [NW, OSH] and
  shipped as uint16 (values fit 16 bits), so the on-chip nibble unpack lands
  directly in [in', out] orientation -- no PE transposes.  Per-(o, group)
  scale/bias are host-expanded to per-word rows (pure broadcast) so dequant
  is four DVE ops per (word-tile, plane): unpack (fused shift+and, u16),
  cast to bf16, t = q * scale, w = t + wbias, written straight into a fully
  SBUF-resident W^T tensor (128 KB/partition) -- no DRAM round-trip.

  Matmul computes y^T = W @ x^T: kxm = W^T (zero-copy views of the resident
  tensor), kxn = x^T bf16 streamed (host-cast, row order in' = ksub*128 + p
  matching the unpack order).  m-tiles align 1:1 with dequant o-quarters, so
  the first quarter unlocks a full ~218 us row of PE work and the rest of
  dequant hides under it.  PSUM partitions carry o, so the output bias is a
  per-partition add at eviction, done on the scalar engine so evictions never
  queue behind the dequant stream in the DVE FIFO (PSUM-bank head-of-line
  blocking).  Host un-transposes y^T.

in' ordering: global k-subtile ksub = wt*4 + plane (wt = 128-word tile of the
packed words, plane = nibble index), so in' = wt*512 + plane*128 + p maps to
original input index 4*(wt*128 + p) + plane.
"""

import numpy as np

B, S, IN, OUT, G = 2, 2048, 4096, 16384, 16
NCORES = 8
OSH = OUT // NCORES       # 2048 out rows per core
BS = B * S                # 4096
NW = IN // 4              # 1024 packed int32 words per out row
P = 128
NKT = IN // 512           # 8 K tiles of 512 (4 k-subtiles each)
NQ = OSH // 512           # 4 o-quarters = matmul m-tiles

_COMPILED = {}


def _build_nc():
    from contextlib import ExitStack

    import concourse.bass as bass
    import concourse.mybir as mybir
    import concourse.tile as tile
    from concourse import bacc
    from concourse.bass import ds, ts
    from concourse.kernels.tile_matmul import (
        ShapeInfo,
        composable_matmul_tile_kernel,
        dma_from_dram_kxn,
        dma_to_dram_mxn,
    )

    f32 = mybir.dt.float32
    bf16 = mybir.dt.bfloat16
    u16 = mybir.dt.uint16

    nc = bacc.Bacc(None, target_bir_lowering=False)

    xtp = nc.dram_tensor("xtp", [IN, BS], bf16, kind="ExternalInput")
    wpkT = nc.dram_tensor("wpkT", [NW, OSH], u16, kind="ExternalInput")
    scx = nc.dram_tensor("scx", [NW, OSH], bf16, kind="ExternalInput")
    wbx = nc.dram_tensor("wbx", [NW, OSH], bf16, kind="ExternalInput")
    bias = nc.dram_tensor("bias", [P, OSH // P], f32, kind="ExternalInput")
    yT = nc.dram_tensor("yT", [OSH, BS], f32, kind="ExternalOutput")

    with tile.TileContext(nc) as tc:
        with ExitStack() as ctx:
            const = ctx.enter_context(tc.tile_pool(name="const", bufs=1))
            dq = ctx.enter_context(tc.tile_pool(name="dq", bufs=3))

            # output bias, per-partition: bias_sb[p, j] = bias[j*128 + p]
            bias_sb = const.tile([P, OSH // P], f32)
            nc.sync.dma_start(bias_sb[:], bias[:])

            # W^T fully resident in SBUF (128 KB/partition): [p, kt, ksub, o]
            wt_res = const.tile([P, NKT, 4, OSH], bf16)

            # ---- Stage 1: dequant (o-quarter-major = matmul m-tile order) ----
            for j in range(NQ):
                osl = ts(j, 512)
                for wt in range(NKT):
                    rsl = ts(wt, P)
                    t_pk = dq.tile([P, 512], u16, tag="pk")
                    nc.sync.dma_start(t_pk[:], wpkT[rsl, osl])
                    t_sc = dq.tile([P, 512], bf16, tag="sc")
                    nc.sync.dma_start(t_sc[:], scx[rsl, osl])
                    t_wb = dq.tile([P, 512], bf16, tag="wb")
                    nc.sync.dma_start(t_wb[:], wbx[rsl, osl])

                    for plane in range(4):
                        q = dq.tile([P, 512], u16, tag="q")
                        nc.vector.tensor_scalar(
                            q[:],
                            t_pk[:],
                            4 * plane,
                            0xF,
                            mybir.AluOpType.logical_shift_right,
                            mybir.AluOpType.bitwise_and,
                        )
                        qf = dq.tile([P, 512], bf16, tag="qf")
                        nc.vector.tensor_copy(qf[:], q[:])
                        t = dq.tile([P, 512], bf16, tag="t")
                        nc.vector.tensor_tensor(
                            t[:], qf[:], t_sc[:], mybir.AluOpType.mult
                        )
                        nc.vector.tensor_tensor(
                            wt_res[:, wt, plane, osl], t[:], t_wb[:], mybir.AluOpType.add
                        )

            # ---- Stage 2: matmul y^T = W @ x^T (+bias at eviction) ----
            kxn_pool = ctx.enter_context(tc.tile_pool(name="kxn", bufs=9))

            kxm_shape = ShapeInfo(pdims=((P, IN // P),), fdims=(OSH,))

            def kxm_producer(nc_, md):
                return wt_res[
                    :, md.k_tile_idx, :, ds(md.m_tile_idx * md.m_tile, md.m_tile)
                ]

            kxn_producer, kxn_shape = dma_from_dram_kxn(kxn_pool, xtp[:])

            def bias_evict(nc_, psum, sbuf, md):
                # On the scalar engine so evictions never queue behind the
                # dequant stream in the DVE FIFO (PSUM-bank head-of-line).
                ob = md.m_tile_idx * 4 + md.m_subtile_idx
                nc_.scalar.activation(
                    sbuf,
                    psum,
                    mybir.ActivationFunctionType.Identity,
                    bias=bias_sb[:, ob : ob + 1],
                    scale=1.0,
                )

            composable_matmul_tile_kernel(
                tc,
                kxm_shape=kxm_shape,
                kxn_shape=kxn_shape,
                output_type=f32,
                kxm_producer=kxm_producer,
                kxn_producer=kxn_producer,
                mxn_consumer=dma_to_dram_mxn(yT[:]),
                mxn_subtile_reducer=bias_evict,
                psum_n_bufs=2,
                temps_n_bufs=2,
            )

    nc.compile()
    return nc


def _get_compiled():
    if "nc" not in _COMPILED:
        _COMPILED["nc"] = _build_nc()
    return _COMPILED["nc"]


def _marshal(input, w_packed, w_scale, w_bias, bias):
    import ml_dtypes

    bf16 = ml_dtypes.bfloat16
    x = np.ascontiguousarray(input, dtype=np.float32).reshape(BS, IN)
    # x^T rows permuted so in' = (wt*4+plane)*128 + p <- original 4*(wt*128+p)+plane
    xt = x.T  # [IN, BS], row index = original in = 4*w + plane, w = wt*128 + p
    xtp = np.ascontiguousarray(
        xt.reshape(NKT, P, 4, BS).transpose(0, 2, 1, 3).reshape(IN, BS).astype(bf16)
    )
    in_maps = []
    for c in range(NCORES):
        osl = slice(c * OSH, (c + 1) * OSH)
        wp = w_packed[osl].reshape(OSH, NW)
        sc = w_scale[osl].reshape(OSH, G).astype(bf16)
        wb = w_bias[osl].reshape(OSH, G).astype(bf16)
        in_maps.append(
            {
                "xtp": xtp,
                "wpkT": np.ascontiguousarray(wp.T.astype(np.uint16)),
                "scx": np.ascontiguousarray(np.repeat(sc.T, NW // G, axis=0)),
                "wbx": np.ascontiguousarray(np.repeat(wb.T, NW // G, axis=0)),
                "bias": np.ascontiguousarray(
                    bias[osl].reshape(OSH // P, P).T, dtype=np.float32
                ),
            }
        )
    return in_maps


def kernel(input, w_packed, w_scale, w_bias, bias, _trace=False, _trace_kwargs=None):
    from concourse.bass_utils import run_bass_kernel_spmd

    nc = _get_compiled()
    in_maps = _marshal(input, w_packed, w_scale, w_bias, bias)
    res = run_bass_kernel_spmd(
        nc,
        in_maps,
        core_ids=list(range(NCORES)),
        trace=_trace,
        **(_trace_kwargs or {}),
    )
    out = np.empty((BS, OUT), dtype=np.float32)
    for c in range(NCORES):
        out[:, c * OSH : (c + 1) * OSH] = res.results[c]["yT"].T
    out = out.reshape(B, S, OUT)
    if _trace:
        return out, res
    return out


# revision 32
# speedup vs baseline: 1.0008x; 1.0008x over previous
"""GroupQuantLinear on 8 Trainium2 NeuronCores.

y[b,s,o] = x[b,s,:] @ W[o,:] + bias[o], where W is dequantized on-device from
4-bit packed weights with per-(o, group) affine scale/bias (groups of 256 along
the 4096-wide input dim).

Sharding: tensor-parallel on out_features (8 shards of 2048 rows); x replicated.

Per-core kernel (Bass/Tile), transpose-free, W-stationary design:
  The packed words are transposed on the HOST (layout-only) to [NW, OSH] and
  shipped as uint16 (values fit 16 bits), so the on-chip nibble unpack lands
  directly in [in', out] orientation -- no PE transposes.  Per-(o, group)
  scale/bias are host-expanded to per-word rows (pure broadcast) so dequant
  is four DVE ops per (word-tile, plane): unpack (fused shift+and, u16),
  cast to bf16, t = q * scale, w = t + wbias, written straight into a fully
  SBUF-resident W^T tensor (128 KB/partition) -- no DRAM round-trip.

  Matmul computes y^T = W @ x^T: kxm = W^T (zero-copy views of the resident
  tensor), kxn = x^T bf16 streamed (host-cast, row order in' = ksub*128 + p
  matching the unpack order).  m-tiles align 1:1 with dequant o-quarters, so
  the first quarter unlocks a full ~218 us row of PE work and the rest of
  dequant hides under it.  PSUM partitions carry o, so the output bias is a
  per-partition add at eviction, done on the scalar engine so evictions never
  queue behind the dequant stream in the DVE FIFO (PSUM-bank head-of-line
  blocking).  Host un-transposes y^T.

in' ordering: global k-subtile ksub = wt*4 + plane (wt = 128-word tile of the
packed words, plane = nibble index), so in' = wt*512 + plane*128 + p maps to
original input index 4*(wt*128 + p) + plane.
"""

import numpy as np

B, S, IN, OUT, G = 2, 2048, 4096, 16384, 16
NCORES = 8
OSH = OUT // NCORES       # 2048 out rows per core
BS = B * S                # 4096
NW = IN // 4              # 1024 packed int32 words per out row
P = 128
NKT = IN // 512           # 8 K tiles of 512 (4 k-subtiles each)
NQ = OSH // 512           # 4 o-quarters = matmul m-tiles

_COMPILED = {}


def _build_nc():
    from contextlib import ExitStack

    import concourse.bass as bass
    import concourse.mybir as mybir
    import concourse.tile as tile
    from concourse import bacc
    from concourse.bass import ds, ts
    from concourse.kernels.tile_matmul import (
        ShapeInfo,
        composable_matmul_tile_kernel,
        dma_from_dram_kxn,
        dma_to_dram_mxn,
    )

    f32 = mybir.dt.float32
    bf16 = mybir.dt.bfloat16
    u16 = mybir.dt.uint16

    nc = bacc.Bacc(None, target_bir_lowering=False)

    xtp = nc.dram_tensor("xtp", [IN, BS], bf16, kind="ExternalInput")
    wpkT = nc.dram_tensor("wpkT", [NW, OSH], u16, kind="ExternalInput")
    scx = nc.dram_tensor("scx", [NW, OSH], bf16, kind="ExternalInput")
    wbx = nc.dram_tensor("wbx", [NW, OSH], bf16, kind="ExternalInput")
    bias = nc.dram_tensor("bias", [P, OSH // P], f32, kind="ExternalInput")
    yT = nc.dram_tensor("yT", [OSH, BS], bf16, kind="ExternalOutput")

    with tile.TileContext(nc) as tc:
        with ExitStack() as ctx:
            const = ctx.enter_context(tc.tile_pool(name="const", bufs=1))
            dq = ctx.enter_context(tc.tile_pool(name="dq", bufs=3))

            # output bias, per-partition: bias_sb[p, j] = bias[j*128 + p]
            bias_sb = const.tile([P, OSH // P], f32)
            nc.sync.dma_start(bias_sb[:], bias[:])

            # W^T fully resident in SBUF (128 KB/partition): [p, kt, ksub, o]
            wt_res = const.tile([P, NKT, 4, OSH], bf16)

            # ---- Stage 1: dequant (o-quarter-major = matmul m-tile order) ----
            for j in range(NQ):
                osl = ts(j, 512)
                for wt in range(NKT):
                    rsl = ts(wt, P)
                    t_pk = dq.tile([P, 512], u16, tag="pk")
                    nc.sync.dma_start(t_pk[:], wpkT[rsl, osl])
                    t_sc = dq.tile([P, 512], bf16, tag="sc")
                    nc.sync.dma_start(t_sc[:], scx[rsl, osl])
                    t_wb = dq.tile([P, 512], bf16, tag="wb")
                    nc.sync.dma_start(t_wb[:], wbx[rsl, osl])

                    for plane in range(4):
                        q = dq.tile([P, 512], u16, tag="q")
                        nc.vector.tensor_scalar(
                            q[:],
                            t_pk[:],
                            4 * plane,
                            0xF,
                            mybir.AluOpType.logical_shift_right,
                            mybir.AluOpType.bitwise_and,
                        )
                        qf = dq.tile([P, 512], bf16, tag="qf")
                        nc.vector.tensor_copy(qf[:], q[:])
                        t = dq.tile([P, 512], bf16, tag="t")
                        nc.vector.tensor_tensor(
                            t[:], qf[:], t_sc[:], mybir.AluOpType.mult
                        )
                        nc.vector.tensor_tensor(
                            wt_res[:, wt, plane, osl], t[:], t_wb[:], mybir.AluOpType.add
                        )

            # ---- Stage 2: matmul y^T = W @ x^T (+bias at eviction) ----
            kxn_pool = ctx.enter_context(tc.tile_pool(name="kxn", bufs=9))

            kxm_shape = ShapeInfo(pdims=((P, IN // P),), fdims=(OSH,))

            def kxm_producer(nc_, md):
                return wt_res[
                    :, md.k_tile_idx, :, ds(md.m_tile_idx * md.m_tile, md.m_tile)
                ]

            kxn_producer, kxn_shape = dma_from_dram_kxn(kxn_pool, xtp[:])

            def bias_evict(nc_, psum, sbuf, md):
                # On the scalar engine so evictions never queue behind the
                # dequant stream in the DVE FIFO (PSUM-bank head-of-line).
                ob = md.m_tile_idx * 4 + md.m_subtile_idx
                nc_.scalar.activation(
                    sbuf,
                    psum,
                    mybir.ActivationFunctionType.Identity,
                    bias=bias_sb[:, ob : ob + 1],
                    scale=1.0,
                )

            composable_matmul_tile_kernel(
                tc,
                kxm_shape=kxm_shape,
                kxn_shape=kxn_shape,
                output_type=bf16,
                kxm_producer=kxm_producer,
                kxn_producer=kxn_producer,
                mxn_consumer=dma_to_dram_mxn(yT[:]),
                mxn_subtile_reducer=bias_evict,
                psum_n_bufs=2,
                temps_n_bufs=2,
            )

    nc.compile()
    return nc


def _get_compiled():
    if "nc" not in _COMPILED:
        _COMPILED["nc"] = _build_nc()
    return _COMPILED["nc"]


def _marshal(input, w_packed, w_scale, w_bias, bias):
    import ml_dtypes

    bf16 = ml_dtypes.bfloat16
    x = np.ascontiguousarray(input, dtype=np.float32).reshape(BS, IN)
    # x^T rows permuted so in' = (wt*4+plane)*128 + p <- original 4*(wt*128+p)+plane
    xt = x.T  # [IN, BS], row index = original in = 4*w + plane, w = wt*128 + p
    xtp = np.ascontiguousarray(
        xt.reshape(NKT, P, 4, BS).transpose(0, 2, 1, 3).reshape(IN, BS).astype(bf16)
    )
    in_maps = []
    for c in range(NCORES):
        osl = slice(c * OSH, (c + 1) * OSH)
        wp = w_packed[osl].reshape(OSH, NW)
        sc = w_scale[osl].reshape(OSH, G).astype(bf16)
        wb = w_bias[osl].reshape(OSH, G).astype(bf16)
        in_maps.append(
            {
                "xtp": xtp,
                "wpkT": np.ascontiguousarray(wp.T.astype(np.uint16)),
                "scx": np.ascontiguousarray(np.repeat(sc.T, NW // G, axis=0)),
                "wbx": np.ascontiguousarray(np.repeat(wb.T, NW // G, axis=0)),
                "bias": np.ascontiguousarray(
                    bias[osl].reshape(OSH // P, P).T, dtype=np.float32
                ),
            }
        )
    return in_maps


def kernel(input, w_packed, w_scale, w_bias, bias, _trace=False, _trace_kwargs=None):
    from concourse.bass_utils import run_bass_kernel_spmd

    nc = _get_compiled()
    in_maps = _marshal(input, w_packed, w_scale, w_bias, bias)
    res = run_bass_kernel_spmd(
        nc,
        in_maps,
        core_ids=list(range(NCORES)),
        trace=_trace,
        **(_trace_kwargs or {}),
    )
    out = np.empty((BS, OUT), dtype=np.float32)
    for c in range(NCORES):
        out[:, c * OSH : (c + 1) * OSH] = res.results[c]["yT"].T.astype(np.float32)
    out = out.reshape(B, S, OUT)
    if _trace:
        return out, res
    return out
